# revision 1
# baseline (speedup 1.0000x reference)
"""Trainium2 Bass kernel for nn_GatedAttentionUnit.

Reference computation (B=4, L=2048, HID=512, PROJ=1024, ATTN=128):
    gva = silu(node @ w1 + b1)                       # [B, L, 2P+A]
    gates, values, base = split(gva, [P, 2P])
    qk = base[..., None, :] * ms_weight + ms_bias    # [B, L, 2, A]
    qk = rope(qk)  (over sequence dim)
    q, k = qk[..., 0, :], qk[..., 1, :]
    logits = einsum('bid,bjd->bij', q * scaling, k) + bias
    attn = softmax(logits, -1)
    out = einsum('bij,bjd->bid', attn, values)
    return (out * gates) @ w2 + b2

Sharding: 8 cores = (batch b in 0..3) x (query-row half h in 0..1).  Each
core computes output rows [h*1024, (h+1)*1024) of batch b with no
cross-core communication: k/values are computed for all 2048 rows of the
batch (duplicated across the 2 cores of a batch pair, ~15% extra flops),
q/gates only for the core's own rows.

On-chip layouts (partition dim first):
    nodeT   [HID, L]      hid on partitions (4 chunks) -> host pre-transposed
    values  [L, PROJ]     rows on partitions (16 chunks)
    gatesT  [PROJ, LH]    proj on partitions (8 chunks)
    kT, qT  [ATTN, *]     head dim on partitions
    logitsT [L, LH]       key rows j on partitions -> softmax sum over
                          partitions via ones-matmul, exp'd logitsT is
                          directly the lhsT for the attn @ values matmul.
RoPE pairs (d, d+64) live on different partitions; DVE ops are
lane-locked, so the rotated copy is produced by a second base projection
whose w1 columns were permuted on the host (SiLU is elementwise, so
silu(shuffle(pre)) == shuffle(silu(pre))).  ms_weight and scaling are
folded into host-built rope tables (rope is linear).

b1/ms_bias are structurally zero in the reference's setup_inputs
(jnp.zeros) and asserted so; b2 is added on the host.

All matmuls run the PE in float32r mode (full rate at free-dim >= 256).
"""

import numpy as np
import sys

try:
    import concourse.bass as bass
except ImportError:  # pragma: no cover
    sys.path.insert(0, "/opt/trn_rl_repo")
    import concourse.bass as bass

import concourse.mybir as mybir
import concourse.tile as tile
from concourse import bacc
from concourse.bass_utils import run_bass_kernel_spmd
from contextlib import ExitStack

B, L, HID, PROJ, ATTN = 4, 2048, 512, 1024, 128
LH = L // 2          # own query rows per core
IH = 512             # i-half processed per attention pass
P = 128
HC = HID // P        # 4 hid chunks
RC = L // P          # 16 row chunks
PC = PROJ // P       # 8 proj chunks
F32 = mybir.dt.float32
F32R = mybir.dt.float32r
AF = mybir.ActivationFunctionType
OP = mybir.AluOpType

_cache = {}


def _build_program():
    nc = bacc.Bacc("TRN2", target_bir_lowering=False, debug=False, num_devices=8)

    dram = {}
    def din(name, shape, dt=F32):
        dram[name] = nc.dram_tensor(name, shape, dt, kind="ExternalInput").ap()
    # float32r inputs: consumed by the PE in fp32r mode (PE rounds
    # internally; bits on the wire are plain fp32)
    din("nodeT", [HID, L], F32R)
    din("nodeTo", [HID, LH], F32R)
    din("biasTo", [L, LH])
    din("w1g", [HID, PROJ], F32R)
    din("w1v", [HID, PROJ], F32R)
    din("w1b", [HID, ATTN], F32R)
    din("w1bs", [HID, ATTN], F32R)
    din("w2", [PROJ, HID], F32R)
    din("Cq", [ATTN, LH])
    din("Sq", [ATTN, LH])
    din("Ck", [ATTN, L])
    din("Sk", [ATTN, L])
    din("onesd", [P, P], F32R)
    out_d = nc.dram_tensor("o", [LH, HID], F32, kind="ExternalOutput").ap()

    def mm(ps, lhsT, rhs, start, stop):
        nc.tensor.matmul(ps, lhsT, rhs, start=start, stop=stop)

    with tile.TileContext(nc) as tc, ExitStack() as top:
        persist = top.enter_context(tc.tile_pool(name="persist", bufs=1))

        kT = persist.tile([P, L], F32R, tag="kT", name="kT")
        qT = persist.tile([P, LH], F32R, tag="qT", name="qT")
        values = [persist.tile([P, PROJ], F32R, tag=f"val{rc}", name=f"val{rc}") for rc in range(RC)]
        gatesT = [persist.tile([P, PROJ // PC * 8], F32R, tag=f"gat{pc}", name=f"gat{pc}")
                  for pc in range(PC)]  # [128, 1024] each (free dim = LH)
        # ---------------- phase 1: projections + rope ------------------------
        with ExitStack() as ph1:
            nodp = ph1.enter_context(tc.tile_pool(name="nod", bufs=1))
            ps_main = ph1.enter_context(tc.tile_pool(name="psm", bufs=2, space="PSUM"))

            nT = [nodp.tile([P, L], F32R, tag=f"nT{hc}", name=f"nT{hc}") for hc in range(HC)]
            nTo = [nodp.tile([P, LH], F32R, tag=f"nTo{hc}", name=f"nTo{hc}") for hc in range(HC)]
            for hc in range(HC):
                nc.sync.dma_start(nT[hc][:], dram["nodeT"][hc * P:(hc + 1) * P, :])
                nc.scalar.dma_start(nTo[hc][:], dram["nodeTo"][hc * P:(hc + 1) * P, :])

            # --- phase 1a: base projections + rope -> kT, qT (scoped) --------
            with ExitStack() as phA:
                wbp = phA.enter_context(tc.tile_pool(name="wb", bufs=1))
                tabp = phA.enter_context(tc.tile_pool(name="tab", bufs=1))
                xp = phA.enter_context(tc.tile_pool(name="xp", bufs=1))

                wball = wbp.tile([P, 2 * HC * ATTN], F32R, tag="wball", name="wball")
                for hc in range(HC):
                    nc.gpsimd.dma_start(wball[:, hc * ATTN:(hc + 1) * ATTN],
                                        dram["w1b"][hc * P:(hc + 1) * P, :])
                    nc.gpsimd.dma_start(wball[:, (HC + hc) * ATTN:(HC + hc + 1) * ATTN],
                                        dram["w1bs"][hc * P:(hc + 1) * P, :])
                w1b = [wball[:, hc * ATTN:(hc + 1) * ATTN] for hc in range(HC)]
                w1bs = [wball[:, (HC + hc) * ATTN:(HC + hc + 1) * ATTN] for hc in range(HC)]
                Cq = tabp.tile([P, LH], F32, tag="Cq", name="Cq")
                Sq = tabp.tile([P, LH], F32, tag="Sq", name="Sq")
                Ck = tabp.tile([P, L], F32, tag="Ck", name="Ck")
                Sk = tabp.tile([P, L], F32, tag="Sk", name="Sk")
                for nm, t in (("Cq", Cq), ("Sq", Sq), ("Ck", Ck), ("Sk", Sk)):
                    nc.gpsimd.dma_start(t[:], dram[nm][:])

                # silu(base): plain variant straight into kT/qT storage,
                # shuffled variant into a shared temp; rope applied in place
                # per 1024-col chunk: dst = dst*C + silu_shuf*S.
                # jobs: (dst slice [P, LH], src tiles, src col offset, C, S slices)
                jobs = [
                    (kT[:, 0:LH],    nT,  0,  Ck[:, 0:LH],  Sk[:, 0:LH]),
                    (kT[:, LH:L],    nT,  LH, Ck[:, LH:L],  Sk[:, LH:L]),
                    (qT[:, 0:LH],    nTo, 0,  Cq[:, 0:LH],  Sq[:, 0:LH]),
                ]
                for dst, srcs, s0, Ct, St in jobs:
                    for w, ev in ((w1b, dst), (w1bs, None)):
                        if ev is None:
                            ev = xp.tile([P, LH], F32R, tag="xsh", name="xsh")
                            xsh = ev
                        for nb in range(2):
                            ps = ps_main.tile([P, 512], F32, tag="ps1", name="ps1")
                            for hc in range(HC):
                                mm(ps, w[hc],
                                   srcs[hc][:, s0 + nb * 512:s0 + (nb + 1) * 512],
                                   start=(hc == 0), stop=(hc == HC - 1))
                            nc.scalar.activation(ev[:, nb * 512:(nb + 1) * 512],
                                                 ps[:], AF.Silu)
                    nc.vector.tensor_tensor(dst, dst, Ct, OP.mult)
                    nc.vector.tensor_tensor(xsh[:], xsh[:], St, OP.mult)
                    nc.vector.tensor_tensor(dst, dst, xsh[:], OP.add)

            # ------------- phase 1b: values [rows, proj] ----------------------
            with ExitStack() as phB:
                wvp = phB.enter_context(tc.tile_pool(name="wv", bufs=1))
                w1v = [wvp.tile([P, PROJ], F32R, tag=f"w1v{hc}", name=f"w1v{hc}") for hc in range(HC)]
                for hc in range(HC):
                    nc.sync.dma_start(w1v[hc][:], dram["w1v"][hc * P:(hc + 1) * P, :])
                for rc in range(RC):
                    for nb in range(PROJ // 512):
                        ps = ps_main.tile([P, 512], F32, tag="ps1", name="ps1")
                        for hc in range(HC):
                            mm(ps, nT[hc][:, rc * P:(rc + 1) * P],
                               w1v[hc][:, nb * 512:(nb + 1) * 512],
                               start=(hc == 0), stop=(hc == HC - 1))
                        nc.scalar.activation(values[rc][:, nb * 512:(nb + 1) * 512],
                                             ps[:], AF.Silu)

            # ------------- phase 1c: gatesT [proj, own rows] ------------------
            with ExitStack() as phC:
                wgp = phC.enter_context(tc.tile_pool(name="wg", bufs=1))
                w1g = [wgp.tile([P, PROJ], F32R, tag=f"w1g{hc}", name=f"w1g{hc}") for hc in range(HC)]
                for hc in range(HC):
                    nc.scalar.dma_start(w1g[hc][:], dram["w1g"][hc * P:(hc + 1) * P, :])
                for pc in range(PC):
                    for nb in range(LH // 512):
                        ps = ps_main.tile([P, 512], F32, tag="ps1", name="ps1")
                        for hc in range(HC):
                            mm(ps, w1g[hc][:, pc * P:(pc + 1) * P],
                               nTo[hc][:, nb * 512:(nb + 1) * 512],
                               start=(hc == 0), stop=(hc == HC - 1))
                        nc.scalar.activation(gatesT[pc][:, nb * 512:(nb + 1) * 512],
                                             ps[:], AF.Silu)

        # w2 resident for phase 2 (loaded after phase-1 pools free their space)
        w2p = top.enter_context(tc.tile_pool(name="w2p", bufs=1))
        w2all = w2p.tile([P, PC * HID], F32R, tag="w2all", name="w2all")
        for pc in range(PC):
            nc.gpsimd.dma_start(w2all[:, pc * HID:(pc + 1) * HID],
                                dram["w2"][pc * P:(pc + 1) * P, :])

        # ---------------- phase 2: attention, per i-half ----------------------
        for hf in range(LH // IH):
            i0 = hf * IH
            with ExitStack() as ph:
                ep = ph.enter_context(tc.tile_pool(name=f"exp{hf}", bufs=1))
                bp = ph.enter_context(tc.tile_pool(name=f"bias{hf}", bufs=2))
                tp = ph.enter_context(tc.tile_pool(name=f"tmp{hf}", bufs=1))
                gp = ph.enter_context(tc.tile_pool(name=f"gated{hf}", bufs=1))
                psl = ph.enter_context(tc.tile_pool(name=f"psl{hf}", bufs=2, space="PSUM"))
                psd = ph.enter_context(tc.tile_pool(name=f"psd{hf}", bufs=1, space="PSUM"))
                pso = ph.enter_context(tc.tile_pool(name=f"pso{hf}", bufs=2, space="PSUM"))

                ones = tp.tile([P, P], F32R, tag="ones", name="ones")
                nc.sync.dma_start(ones[:], dram["onesd"][:])
                # expT packed 2 j-chunks per tile along free dim
                exp2 = [ep.tile([P, 2 * IH], F32R, tag=f"e{jj}", name=f"e{jj}")
                        for jj in range(RC // 2)]
                expT = [exp2[jc // 2][:, (jc % 2) * IH:(jc % 2 + 1) * IH]
                        for jc in range(RC)]
                # logitsT chunk -> +bias -> exp
                for jc in range(RC):
                    ps = psl.tile([P, IH], F32, tag="pslg", name="pslg", bufs=2)
                    mm(ps, kT[:, jc * P:(jc + 1) * P], qT[:, i0:i0 + IH],
                       start=True, stop=True)
                    bt = bp.tile([P, IH], F32, tag="bt", name="bt")
                    nc.scalar.dma_start(
                        bt[:], dram["biasTo"][jc * P:(jc + 1) * P, i0:i0 + IH])
                    nc.vector.tensor_tensor(ps[:], ps[:], bt[:], OP.add)
                    nc.scalar.activation(expT[jc], ps[:], AF.Exp)
                # denominator, replicated across partitions via ones-matmul
                psn = psd.tile([P, IH], F32, tag="psden", name="psden")
                for jc in range(RC):
                    mm(psn, ones[:], expT[jc], start=(jc == 0), stop=(jc == RC - 1))
                recipR = tp.tile([P, IH], F32, tag="recip", name="recip")
                nc.vector.reciprocal(recipR[:], psn[:])
                # attn @ values (transposed) + normalize + gate;
                # gated packed 2 p-chunks per tile along free dim
                gated2 = [gp.tile([P, 2 * IH], F32R, tag=f"g{k}", name=f"g{k}")
                          for k in range(PC // 2)]
                for pc in range(PC):
                    ps = pso.tile([P, IH], F32, tag="psov", name="psov", bufs=2)
                    for jc in range(RC):
                        mm(ps, values[jc][:, pc * P:(pc + 1) * P], expT[jc],
                           start=(jc == 0), stop=(jc == RC - 1))
                    gslot = gated2[pc // 2][:, (pc % 2) * IH:(pc % 2 + 1) * IH]
                    nc.vector.tensor_tensor(gslot, ps[:], recipR[:], OP.mult)
                    nc.vector.tensor_tensor(gslot, gslot,
                                            gatesT[pc][:, i0:i0 + IH], OP.mult)
                # output projection
                for ic in range(IH // P):
                    ps = pso.tile([P, HID], F32, tag="psf", name="psf")
                    for pc in range(PC):
                        mm(ps, gated2[pc // 2][:, (pc % 2) * IH + ic * P:(pc % 2) * IH + (ic + 1) * P],
                           w2all[:, pc * HID:(pc + 1) * HID],
                           start=(pc == 0), stop=(pc == PC - 1))
                    osb = tp.tile([P, HID], F32, tag="osb", name="osb", bufs=2)
                    nc.scalar.copy(osb[:], ps[:])
                    r0 = i0 + ic * P
                    nc.scalar.dma_start(out_d[r0:r0 + P, :], osb[:])

    nc.compile()
    return nc


def _rope_tables(ms_weight, scaling):
    half = ATTN // 2
    inv_freq = np.power(10000.0, -np.arange(half, dtype=np.float32) / half)
    pos = np.arange(L, dtype=np.float32)
    sinusoid = pos[:, None] * inv_freq[None, :]          # [L, half]
    sinT = np.sin(sinusoid).T.astype(np.float32)         # [half, L]
    cosT = np.cos(sinusoid).T.astype(np.float32)

    def tables(m):
        m1, m2 = m[:half, None], m[half:, None]
        C = np.concatenate([cosT * m1, cosT * m2], axis=0)
        S = np.concatenate([-sinT * m2, sinT * m1], axis=0)
        return np.ascontiguousarray(C), np.ascontiguousarray(S)

    mq = (ms_weight[0] * np.float32(scaling[0])).astype(np.float32)
    mk = ms_weight[1].astype(np.float32)
    Cq, Sq = tables(mq)
    Ck, Sk = tables(mk)
    return Cq, Sq, Ck, Sk


def kernel(node, bias, scaling, w1, b1, ms_weight, ms_bias, w2, b2):
    assert np.abs(b1).max() == 0.0 and np.abs(ms_bias).max() == 0.0, \
        "kernel assumes b1/ms_bias are zero (as in reference setup_inputs)"

    if "nc" not in _cache:
        _cache["nc"] = _build_program()
    nc = _cache["nc"]

    node = np.asarray(node, np.float32)
    bias = np.asarray(bias, np.float32)
    w1 = np.asarray(w1, np.float32)
    w2c = np.ascontiguousarray(np.asarray(w2, np.float32))

    nodeT = np.ascontiguousarray(node.transpose(0, 2, 1))          # [B, HID, L]
    biasT = np.ascontiguousarray(bias.transpose(0, 2, 1))          # [B, L(j), L(i)]
    shuf = (np.arange(ATTN) + ATTN // 2) % ATTN
    w1g = np.ascontiguousarray(w1[:, :PROJ])
    w1v = np.ascontiguousarray(w1[:, PROJ:2 * PROJ])
    w1b = np.ascontiguousarray(w1[:, 2 * PROJ:])
    w1bs = np.ascontiguousarray(w1b[:, shuf])
    CqF, SqF, Ck, Sk = _rope_tables(np.asarray(ms_weight, np.float32),
                                    np.asarray(scaling, np.float32))

    ones_np = np.ones((P, P), np.float32)
    in_maps = []
    for c in range(8):
        b, h = c // 2, c % 2
        sl = slice(h * LH, (h + 1) * LH)
        in_maps.append({
            "nodeT": nodeT[b],
            "nodeTo": np.ascontiguousarray(nodeT[b][:, sl]),
            "biasTo": np.ascontiguousarray(biasT[b][:, sl]),
            "w1g": w1g, "w1v": w1v, "w1b": w1b, "w1bs": w1bs,
            "w2": w2c,
            "Cq": np.ascontiguousarray(CqF[:, sl]),
            "Sq": np.ascontiguousarray(SqF[:, sl]),
            "Ck": Ck, "Sk": Sk,
            "onesd": ones_np,
        })

    res = run_bass_kernel_spmd(nc, in_maps, list(range(8)))
    out = np.empty((B, L, HID), np.float32)
    for c in range(8):
        b, h = c // 2, c % 2
        out[b, h * LH:(h + 1) * LH, :] = res.results[c]["o"]
    out += np.asarray(b2, np.float32)[None, None, :]
    return out

